# revision 9
# baseline (speedup 1.0000x reference)
"""Trainium2 Bass kernel for the Adalibi histogram-binning problem.

out[n, 0, h, c] = A[h] * (c == inv0[n])            for c in [0, 2048)
                + A[h] * (c - 2048 == inv1[n])     for c in [2048, 4096)

where inv_p[n] is the cumulative count of ceil-bin changes of
idx_p[n] = ceil((t[n] - u_p) / delta_p), t[n] = k + n, and
A[h] = sqrt(exp(slope_h)) / sqrt(2).

Sharding: rows n are split across 8 NeuronCores (256 rows each). Every core
redundantly computes the tiny global bin-change vector (the divisions/ceils
for all 2048 positions, laid out n = p + 128*s on 128 partitions x 16
chunks), then extracts its own 256 inv values with two small matmuls
(all-ones prefix broadcast + lower-triangular prefix), and finally generates
and writes its 64 MiB output shard (memory-bound).

Bit-exactness with the f32 reference:
  - division: HW reciprocal seed + one exact-residual correction (Veltkamp
    split products) -> correctly-rounded f32 quotient.
  - ceil: round-to-nearest via +-2^23, then +1 where rounded < value.
  - neighbour bin index idx[n-1] is recomputed from (t[n]-1) (exact integer)
    instead of shifting across partitions.
"""

import math
from contextlib import ExitStack

import numpy as np

N = 2048          # seq_len
NN = 2 * N        # output columns (P*N)
H = 16            # heads
NCORES = 8
ROWS = N // NCORES  # 256 rows per core
KOFF = 37
S = 16            # n-chunks of 128: n = p + 128*s
MAGIC = 8388608.0  # 2^23
SPLIT = 4097.0     # 2^12 + 1 Veltkamp constant
HGRP = 2          # heads per output tile
OUT_BUFS = 4


def get_slopes(n):
    def pow2(m):
        start = 2 ** (-(2 ** (-(math.log2(m) - 3))))
        return [start * start**i for i in range(m)]

    if math.log2(n).is_integer():
        return pow2(n)
    c = 2 ** math.floor(math.log2(n))
    return pow2(c) + get_slopes(2 * c)[0::2][: n - c]


def _amps():
    # mimic reference f32 op order: sqrt(exp(slopes_f32)) * (1/sqrt(2))
    slopes = np.asarray(get_slopes(H), dtype=np.float32)
    sq = np.sqrt(np.exp(slopes)).astype(np.float32)
    z = np.float32(1.0) / np.sqrt(np.float32(2.0))
    return [float(np.float32(a) * z) for a in sq]


def _host_consts():
    # tmat[p, 16*g + s]: stream g in {a0, b0, a1, b1}; value t = KOFF + p + 128*s
    # (minus 1 for the b streams).
    p = np.arange(128).reshape(-1, 1)
    s = np.arange(S).reshape(1, -1)
    t = (KOFF + p + 128 * s).astype(np.float32)  # (128, 16)
    tmat = np.concatenate([t, t - 1, t, t - 1], axis=1).astype(np.float32)

    # bsel[k, j]: broadcast-selector. vals partitions: [d0, d1, u0, u1].
    # cols 0:64 -> D tile groups [d0,d0,d1,d1]; cols 64:128 -> U tile.
    bsel = np.zeros((4, 128), dtype=np.float32)
    for j in range(64):
        bsel[j // 32, j] = 1.0
        bsel[2 + j // 32, 64 + j] = 1.0

    # tri[p, q] = 1 if p <= q (inclusive prefix within the active chunk)
    tri = np.tril(np.ones((128, 128), dtype=np.float32)).T.copy()
    return tmat, bsel, tri


def _wsel_for_core(c):
    # cols 0:16 = W_0 (s < 2c), 16:32 = W_1 (s < 2c+1),
    # 32:48 = sel_0 (s == 2c), 48:64 = sel_1 (s == 2c+1); rows identical.
    w = np.zeros((128, 64), dtype=np.float32)
    s = np.arange(S)
    for j in range(2):
        w[:, j * 16:(j + 1) * 16] = (s < 2 * c + j).astype(np.float32)[None, :]
        w[:, 32 + j * 16:48 + j * 16] = (s == 2 * c + j).astype(np.float32)[None, :]
    return w


_NC = None


def _build():
    import concourse.bacc as bacc
    import concourse.mybir as mybir
    from concourse.tile import TileContext
    from concourse.alu_op_type import AluOpType as alu

    f32 = mybir.dt.float32
    i32 = mybir.dt.int32
    nc = bacc.Bacc("TRN2")

    duv_d = nc.dram_tensor("duv", (4, 1), f32, kind="ExternalInput")
    tmat_d = nc.dram_tensor("tmat", (128, 64), f32, kind="ExternalInput")
    bsel_d = nc.dram_tensor("bsel", (4, 128), f32, kind="ExternalInput")
    tri_d = nc.dram_tensor("tri", (128, 128), f32, kind="ExternalInput")
    wsel_d = nc.dram_tensor("wsel", (128, 64), f32, kind="ExternalInput")
    out_d = nc.dram_tensor("out", (ROWS, H, NN), f32, kind="ExternalOutput")

    amps = _amps()

    with TileContext(nc) as tc:
        with ExitStack() as ctx:
            const = ctx.enter_context(tc.tile_pool(name="const", bufs=1))
            work = ctx.enter_context(tc.tile_pool(name="work", bufs=1))
            psum = ctx.enter_context(tc.tile_pool(name="psum", bufs=1, space="PSUM"))
            outp = ctx.enter_context(tc.tile_pool(name="outp", bufs=OUT_BUFS))

            # ---- load constants / inputs --------------------------------
            tmat = const.tile([128, 64], f32)
            nc.sync.dma_start(out=tmat[:, :], in_=tmat_d[:, :])
            bselt = const.tile([4, 128], f32)
            nc.sync.dma_start(out=bselt[:, :], in_=bsel_d[:, :])
            trit = const.tile([128, 128], f32)
            nc.sync.dma_start(out=trit[:, :], in_=tri_d[:, :])
            wselt = const.tile([128, 64], f32)
            nc.sync.dma_start(out=wselt[:, :], in_=wsel_d[:, :])
            vals = const.tile([4, 1], f32)
            nc.sync.dma_start(out=vals[:, :], in_=duv_d[:, :])

            # ---- broadcast d/u to all partitions via PE -----------------
            ones4 = const.tile([4, 128], f32)
            nc.vector.memset(ones4[:, :], 1.0)
            lmat = work.tile([4, 128], f32)
            nc.vector.tensor_tensor(
                out=lmat[:, :], in0=ones4[:, :],
                in1=vals[:, 0:1].to_broadcast((4, 128)), op=alu.mult)
            du_ps = psum.tile([128, 128], f32, tag="du_ps")
            nc.tensor.matmul(du_ps[:, :], lhsT=lmat[:, :], rhs=bselt[:, :],
                             start=True, stop=True)
            DU = work.tile([128, 128], f32)
            nc.vector.tensor_copy(out=DU[:, :], in_=du_ps[:, :])
            D = DU[:, 0:64]
            U = DU[:, 64:128]

            # ---- exact f32 division q = (t - u) / d ---------------------
            def tt(name, a, b, op):
                o = work.tile([128, 64], f32, tag=name)
                nc.vector.tensor_tensor(out=o[:, :], in0=a, in1=b, op=op)
                return o[:, :]

            def ts(name, a, s1, op):
                o = work.tile([128, 64], f32, tag=name)
                nc.vector.tensor_scalar(out=o[:, :], in0=a, scalar1=s1,
                                        scalar2=None, op0=op)
                return o[:, :]

            X = tt("X", tmat[:, :], U, alu.subtract)
            r = work.tile([128, 64], f32)
            nc.vector.reciprocal(out=r[:, :], in_=D)
            q0 = tt("q0", X, r[:, :], alu.mult)
            c1 = ts("c1", q0, SPLIT, alu.mult)
            t2 = tt("t2", c1, q0, alu.subtract)
            hq = tt("hq", c1, t2, alu.subtract)
            lq = tt("lq", q0, hq, alu.subtract)
            c2 = ts("c2", D, SPLIT, alu.mult)
            t3 = tt("t3", c2, D, alu.subtract)
            hd = tt("hd", c2, t3, alu.subtract)
            ld = tt("ld", D, hd, alu.subtract)
            phh = tt("phh", hq, hd, alu.mult)
            phl = tt("phl", hq, ld, alu.mult)
            plh = tt("plh", lq, hd, alu.mult)
            pll = tt("pll", lq, ld, alu.mult)
            e1 = tt("e1", X, phh, alu.subtract)
            e2 = tt("e2", e1, phl, alu.subtract)
            e3 = tt("e3", e2, plh, alu.subtract)
            e4 = tt("e4", e3, pll, alu.subtract)
            corr = tt("corr", e4, r[:, :], alu.mult)
            q = tt("q", q0, corr, alu.add)

            # ---- ceil + change bits -------------------------------------
            y1 = ts("y1", q, MAGIC, alu.add)
            y = ts("y", y1, MAGIC, alu.subtract)
            g = tt("g", q, y, alu.is_gt)
            ce = tt("ce", y, g, alu.add)
            ch = work.tile([128, 32], f32)
            nc.vector.tensor_tensor(out=ch[:, 0:16], in0=ce[:, 0:16],
                                    in1=ce[:, 16:32], op=alu.not_equal)
            nc.vector.tensor_tensor(out=ch[:, 16:32], in0=ce[:, 32:48],
                                    in1=ce[:, 48:64], op=alu.not_equal)
            nc.vector.memset(ch[0:1, 0:1], 0.0)   # change[0] := 0 (block 0)
            nc.vector.memset(ch[0:1, 16:17], 0.0)  # change[0] := 0 (block 1)

            # ---- per-core inv targets via prefix matmuls ----------------
            ones128 = const.tile([128, 128], f32)
            nc.vector.memset(ones128[:, :], 1.0)
            tgts = work.tile([128, 4], f32)  # col 2*j + b
            for b in range(2):
                for j in range(2):
                    chb = ch[:, 16 * b:16 * b + 16]
                    wj = wselt[:, 16 * j:16 * (j + 1)]
                    sj = wselt[:, 32 + 16 * j:48 + 16 * j]
                    tmpw = work.tile([128, 16], f32, tag=f"tmpw{b}{j}")
                    nc.vector.tensor_tensor(out=tmpw[:, :], in0=chb, in1=wj,
                                            op=alu.mult)
                    chw = work.tile([128, 1], f32, tag=f"chw{b}{j}")
                    nc.vector.tensor_reduce(out=chw[:, 0:1], in_=tmpw[:, :],
                                            axis=mybir.AxisListType.X,
                                            op=alu.add)
                    tmps = work.tile([128, 16], f32, tag=f"tmps{b}{j}")
                    nc.vector.tensor_tensor(out=tmps[:, :], in0=chb, in1=sj,
                                            op=alu.mult)
                    chs = work.tile([128, 1], f32, tag=f"chs{b}{j}")
                    nc.vector.tensor_reduce(out=chs[:, 0:1], in_=tmps[:, :],
                                            axis=mybir.AxisListType.X,
                                            op=alu.add)
                    tps = psum.tile([128, 1], f32, tag=f"tps{b}{j}")
                    nc.tensor.matmul(tps[:, 0:1], lhsT=ones128[:, :],
                                     rhs=chw[:, 0:1], start=True, stop=False)
                    nc.tensor.matmul(tps[:, 0:1], lhsT=trit[:, :],
                                     rhs=chs[:, 0:1], start=False, stop=True)
                    nc.vector.tensor_copy(out=tgts[:, 2 * j + b:2 * j + b + 1],
                                          in_=tps[:, 0:1])

            # ---- column iota --------------------------------------------
            ioti = const.tile([128, N], i32)
            nc.gpsimd.iota(ioti[:, :], pattern=[[1, N]], base=0,
                           channel_multiplier=0)
            iot = const.tile([128, N], f32)
            nc.vector.tensor_copy(out=iot[:, :], in_=ioti[:, :])

            # ---- generate + store output tiles --------------------------
            copyf = mybir.ActivationFunctionType.Copy
            opi = 0
            for j in range(2):
                zrow = work.tile([128, NN], f32, tag=f"zrow{j}")
                nc.vector.tensor_tensor(
                    out=zrow[:, 0:N], in0=iot[:, :],
                    in1=tgts[:, 2 * j:2 * j + 1].to_broadcast((128, N)),
                    op=alu.is_equal)
                nc.vector.tensor_tensor(
                    out=zrow[:, N:NN], in0=iot[:, :],
                    in1=tgts[:, 2 * j + 1:2 * j + 2].to_broadcast((128, N)),
                    op=alu.is_equal)
                for hg in range(H // HGRP):
                    ob = outp.tile([128, HGRP, NN], f32)
                    for hs in range(HGRP):
                        h = hg * HGRP + hs
                        k = opi % 3
                        opi += 1
                        if k == 0:
                            nc.vector.tensor_scalar(
                                out=ob[:, hs, :], in0=zrow[:, :],
                                scalar1=amps[h], scalar2=None, op0=alu.mult)
                        elif k == 1:
                            nc.gpsimd.tensor_scalar(
                                out=ob[:, hs, :], in0=zrow[:, :],
                                scalar1=amps[h], scalar2=None, op0=alu.mult)
                        else:
                            nc.scalar.activation(
                                out=ob[:, hs, :], in_=zrow[:, :],
                                func=copyf, scale=amps[h])
                    nc.sync.dma_start(
                        out=out_d[j * 128:(j + 1) * 128,
                                  hg * HGRP:(hg + 1) * HGRP, :],
                        in_=ob[:, :, :])
    nc.compile()
    return nc


def _get_nc():
    global _NC
    if _NC is None:
        _NC = _build()
    return _NC


def _run(inputs, trace=False, **kw):
    from concourse.bass_utils import run_bass_kernel_spmd

    delta = np.ascontiguousarray(
        np.asarray(inputs["delta"], dtype=np.float32).reshape(2, 1))
    u = np.ascontiguousarray(
        np.asarray(inputs["u"], dtype=np.float32).reshape(2, 1))
    assert int(inputs.get("seq_len", N)) == N
    assert int(inputs.get("k", KOFF)) == KOFF

    nc = _get_nc()
    tmat, bsel, tri = _host_consts()
    duv = np.ascontiguousarray(np.concatenate([delta, u], axis=0))
    in_maps = []
    for c in range(NCORES):
        in_maps.append({
            "duv": duv, "tmat": tmat, "bsel": bsel,
            "tri": tri, "wsel": _wsel_for_core(c),
        })
    res = run_bass_kernel_spmd(nc, in_maps, core_ids=list(range(NCORES)),
                               trace=trace, **kw)
    shards = [res.results[i]["out"] for i in range(NCORES)]
    full = np.concatenate(shards, axis=0)  # (2048, 16, 4096)
    return full[:, None, :, :], res


def kernel(**inputs) -> np.ndarray:
    out, _ = _run(inputs)
    return np.ascontiguousarray(out.astype(np.float32, copy=False))


# revision 10
# speedup vs baseline: 4.4589x; 4.4589x over previous
"""Trainium2 Bass kernel for the Adalibi histogram-binning problem.

out[n, 0, h, c] = A[h] * (c == inv0[n])            for c in [0, 2048)
                + A[h] * (c - 2048 == inv1[n])     for c in [2048, 4096)

where inv_p[n] is the cumulative count of ceil-bin changes of
idx_p[n] = ceil((t[n] - u_p) / delta_p), t[n] = k + n, and
A[h] = sqrt(exp(slope_h)) / sqrt(2).

Sharding: rows n are split across 8 NeuronCores (256 rows each). Every core
redundantly computes the tiny global bin-change vector (the divisions/ceils
for all 2048 positions, laid out n = p + 128*s on 128 partitions x 16
chunks), then extracts its own 256 inv values with two small matmuls
(all-ones prefix broadcast + lower-triangular prefix), and finally generates
and writes its 64 MiB output shard (memory-bound).

Bit-exactness with the f32 reference:
  - division: HW reciprocal seed + one exact-residual correction (Veltkamp
    split products) -> correctly-rounded f32 quotient.
  - ceil: round-to-nearest via +-2^23, then +1 where rounded < value.
  - neighbour bin index idx[n-1] is recomputed from (t[n]-1) (exact integer)
    instead of shifting across partitions.
"""

import math
from contextlib import ExitStack

import numpy as np

N = 2048          # seq_len
NN = 2 * N        # output columns (P*N)
H = 16            # heads
NCORES = 8
ROWS = N // NCORES  # 256 rows per core
KOFF = 37
S = 16            # n-chunks of 128: n = p + 128*s
MAGIC = 8388608.0  # 2^23
SPLIT = 4097.0     # 2^12 + 1 Veltkamp constant
HGRP = 2          # heads per output tile
OUT_BUFS = 4


def get_slopes(n):
    def pow2(m):
        start = 2 ** (-(2 ** (-(math.log2(m) - 3))))
        return [start * start**i for i in range(m)]

    if math.log2(n).is_integer():
        return pow2(n)
    c = 2 ** math.floor(math.log2(n))
    return pow2(c) + get_slopes(2 * c)[0::2][: n - c]


def _amps():
    # mimic reference f32 op order: sqrt(exp(slopes_f32)) * (1/sqrt(2))
    slopes = np.asarray(get_slopes(H), dtype=np.float32)
    sq = np.sqrt(np.exp(slopes)).astype(np.float32)
    z = np.float32(1.0) / np.sqrt(np.float32(2.0))
    return [float(np.float32(a) * z) for a in sq]


def _host_consts():
    # tmat[p, 16*g + s]: stream g in {a0, b0, a1, b1}; value t = KOFF + p + 128*s
    # (minus 1 for the b streams).
    p = np.arange(128).reshape(-1, 1)
    s = np.arange(S).reshape(1, -1)
    t = (KOFF + p + 128 * s).astype(np.float32)  # (128, 16)
    tmat = np.concatenate([t, t - 1, t, t - 1], axis=1).astype(np.float32)

    # bsel[k, j]: broadcast-selector. vals partitions: [d0, d1, u0, u1].
    # cols 0:64 -> D tile groups [d0,d0,d1,d1]; cols 64:128 -> U tile.
    bsel = np.zeros((4, 128), dtype=np.float32)
    for j in range(64):
        bsel[j // 32, j] = 1.0
        bsel[2 + j // 32, 64 + j] = 1.0

    # tri[p, q] = 1 if p <= q (inclusive prefix within the active chunk)
    tri = np.tril(np.ones((128, 128), dtype=np.float32)).T.copy()
    return tmat, bsel, tri


def _wsel_for_core(c):
    # cols 0:16 = W_0 (s < 2c), 16:32 = W_1 (s < 2c+1),
    # 32:48 = sel_0 (s == 2c), 48:64 = sel_1 (s == 2c+1); rows identical.
    w = np.zeros((128, 64), dtype=np.float32)
    s = np.arange(S)
    for j in range(2):
        w[:, j * 16:(j + 1) * 16] = (s < 2 * c + j).astype(np.float32)[None, :]
        w[:, 32 + j * 16:48 + j * 16] = (s == 2 * c + j).astype(np.float32)[None, :]
    return w


_NC = None


def _build():
    import concourse.bacc as bacc
    import concourse.mybir as mybir
    from concourse.tile import TileContext
    from concourse.alu_op_type import AluOpType as alu

    f32 = mybir.dt.float32
    i32 = mybir.dt.int32
    nc = bacc.Bacc("TRN2")

    duv_d = nc.dram_tensor("duv", (4, 1), f32, kind="ExternalInput")
    tmat_d = nc.dram_tensor("tmat", (128, 64), f32, kind="ExternalInput")
    bsel_d = nc.dram_tensor("bsel", (4, 128), f32, kind="ExternalInput")
    tri_d = nc.dram_tensor("tri", (128, 128), f32, kind="ExternalInput")
    wsel_d = nc.dram_tensor("wsel", (128, 64), f32, kind="ExternalInput")
    out_d = nc.dram_tensor("out", (ROWS, H, NN), f32, kind="ExternalOutput")

    amps = _amps()

    with TileContext(nc) as tc:
        with ExitStack() as ctx:
            const = ctx.enter_context(tc.tile_pool(name="const", bufs=1))
            work = ctx.enter_context(tc.tile_pool(name="work", bufs=1))
            psum = ctx.enter_context(tc.tile_pool(name="psum", bufs=1, space="PSUM"))
            outp = ctx.enter_context(tc.tile_pool(name="outp", bufs=OUT_BUFS))

            # ---- load constants / inputs --------------------------------
            tmat = const.tile([128, 64], f32)
            nc.sync.dma_start(out=tmat[:, :], in_=tmat_d[:, :])
            bselt = const.tile([4, 128], f32)
            nc.sync.dma_start(out=bselt[:, :], in_=bsel_d[:, :])
            trit = const.tile([128, 128], f32)
            nc.sync.dma_start(out=trit[:, :], in_=tri_d[:, :])
            wselt = const.tile([128, 64], f32)
            nc.sync.dma_start(out=wselt[:, :], in_=wsel_d[:, :])
            vals = const.tile([4, 1], f32)
            nc.sync.dma_start(out=vals[:, :], in_=duv_d[:, :])

            # ---- broadcast d/u to all partitions via PE -----------------
            ones4 = const.tile([4, 128], f32)
            nc.vector.memset(ones4[:, :], 1.0)
            lmat = work.tile([4, 128], f32)
            nc.vector.tensor_tensor(
                out=lmat[:, :], in0=ones4[:, :],
                in1=vals[:, 0:1].to_broadcast((4, 128)), op=alu.mult)
            du_ps = psum.tile([128, 128], f32, tag="du_ps")
            nc.tensor.matmul(du_ps[:, :], lhsT=lmat[:, :], rhs=bselt[:, :],
                             start=True, stop=True)
            DU = work.tile([128, 128], f32)
            nc.vector.tensor_copy(out=DU[:, :], in_=du_ps[:, :])
            D = DU[:, 0:64]
            U = DU[:, 64:128]

            # ---- exact f32 division q = (t - u) / d ---------------------
            def tt(name, a, b, op):
                o = work.tile([128, 64], f32, tag=name)
                nc.vector.tensor_tensor(out=o[:, :], in0=a, in1=b, op=op)
                return o[:, :]

            def ts(name, a, s1, op):
                o = work.tile([128, 64], f32, tag=name)
                nc.vector.tensor_scalar(out=o[:, :], in0=a, scalar1=s1,
                                        scalar2=None, op0=op)
                return o[:, :]

            X = tt("X", tmat[:, :], U, alu.subtract)
            r = work.tile([128, 64], f32)
            nc.vector.reciprocal(out=r[:, :], in_=D)
            q0 = tt("q0", X, r[:, :], alu.mult)
            c1 = ts("c1", q0, SPLIT, alu.mult)
            t2 = tt("t2", c1, q0, alu.subtract)
            hq = tt("hq", c1, t2, alu.subtract)
            lq = tt("lq", q0, hq, alu.subtract)
            c2 = ts("c2", D, SPLIT, alu.mult)
            t3 = tt("t3", c2, D, alu.subtract)
            hd = tt("hd", c2, t3, alu.subtract)
            ld = tt("ld", D, hd, alu.subtract)
            phh = tt("phh", hq, hd, alu.mult)
            phl = tt("phl", hq, ld, alu.mult)
            plh = tt("plh", lq, hd, alu.mult)
            pll = tt("pll", lq, ld, alu.mult)
            e1 = tt("e1", X, phh, alu.subtract)
            e2 = tt("e2", e1, phl, alu.subtract)
            e3 = tt("e3", e2, plh, alu.subtract)
            e4 = tt("e4", e3, pll, alu.subtract)
            corr = tt("corr", e4, r[:, :], alu.mult)
            q = tt("q", q0, corr, alu.add)

            # ---- ceil + change bits -------------------------------------
            y1 = ts("y1", q, MAGIC, alu.add)
            y = ts("y", y1, MAGIC, alu.subtract)
            g = tt("g", q, y, alu.is_gt)
            ce = tt("ce", y, g, alu.add)
            ch = work.tile([128, 32], f32)
            nc.vector.tensor_tensor(out=ch[:, 0:16], in0=ce[:, 0:16],
                                    in1=ce[:, 16:32], op=alu.not_equal)
            nc.vector.tensor_tensor(out=ch[:, 16:32], in0=ce[:, 32:48],
                                    in1=ce[:, 48:64], op=alu.not_equal)
            nc.vector.memset(ch[0:1, 0:1], 0.0)   # change[0] := 0 (block 0)
            nc.vector.memset(ch[0:1, 16:17], 0.0)  # change[0] := 0 (block 1)

            # ---- per-core inv targets via prefix matmuls ----------------
            ones128 = const.tile([128, 128], f32)
            nc.vector.memset(ones128[:, :], 1.0)
            tgts = work.tile([128, 4], f32)  # col 2*j + b
            for b in range(2):
                for j in range(2):
                    chb = ch[:, 16 * b:16 * b + 16]
                    wj = wselt[:, 16 * j:16 * (j + 1)]
                    sj = wselt[:, 32 + 16 * j:48 + 16 * j]
                    tmpw = work.tile([128, 16], f32, tag=f"tmpw{b}{j}")
                    nc.vector.tensor_tensor(out=tmpw[:, :], in0=chb, in1=wj,
                                            op=alu.mult)
                    chw = work.tile([128, 1], f32, tag=f"chw{b}{j}")
                    nc.vector.tensor_reduce(out=chw[:, 0:1], in_=tmpw[:, :],
                                            axis=mybir.AxisListType.X,
                                            op=alu.add)
                    tmps = work.tile([128, 16], f32, tag=f"tmps{b}{j}")
                    nc.vector.tensor_tensor(out=tmps[:, :], in0=chb, in1=sj,
                                            op=alu.mult)
                    chs = work.tile([128, 1], f32, tag=f"chs{b}{j}")
                    nc.vector.tensor_reduce(out=chs[:, 0:1], in_=tmps[:, :],
                                            axis=mybir.AxisListType.X,
                                            op=alu.add)
                    tps = psum.tile([128, 1], f32, tag=f"tps{b}{j}")
                    nc.tensor.matmul(tps[:, 0:1], lhsT=ones128[:, :],
                                     rhs=chw[:, 0:1], start=True, stop=False)
                    nc.tensor.matmul(tps[:, 0:1], lhsT=trit[:, :],
                                     rhs=chs[:, 0:1], start=False, stop=True)
                    nc.vector.tensor_copy(out=tgts[:, 2 * j + b:2 * j + b + 1],
                                          in_=tps[:, 0:1])

            # ---- column iota --------------------------------------------
            ioti = const.tile([128, N], i32)
            nc.gpsimd.iota(ioti[:, :], pattern=[[1, N]], base=0,
                           channel_multiplier=0)
            iot = const.tile([128, N], f32)
            nc.vector.tensor_copy(out=iot[:, :], in_=ioti[:, :])

            # ---- generate + store output tiles --------------------------
            copyf = mybir.ActivationFunctionType.Copy
            opi = 0
            for j in range(2):
                zrow = work.tile([128, NN], f32, tag=f"zrow{j}")
                nc.vector.tensor_tensor(
                    out=zrow[:, 0:N], in0=iot[:, :],
                    in1=tgts[:, 2 * j:2 * j + 1].to_broadcast((128, N)),
                    op=alu.is_equal)
                nc.vector.tensor_tensor(
                    out=zrow[:, N:NN], in0=iot[:, :],
                    in1=tgts[:, 2 * j + 1:2 * j + 2].to_broadcast((128, N)),
                    op=alu.is_equal)
                for hg in range(H // HGRP):
                    ob = outp.tile([128, HGRP, NN], f32)
                    for hs in range(HGRP):
                        h = hg * HGRP + hs
                        k = opi % 2
                        opi += 1
                        if k == 0:
                            nc.vector.tensor_scalar(
                                out=ob[:, hs, :], in0=zrow[:, :],
                                scalar1=amps[h], scalar2=None, op0=alu.mult)
                        else:
                            nc.scalar.activation(
                                out=ob[:, hs, :], in_=zrow[:, :],
                                func=copyf, scale=amps[h])
                    nc.sync.dma_start(
                        out=out_d[j * 128:(j + 1) * 128,
                                  hg * HGRP:(hg + 1) * HGRP, :],
                        in_=ob[:, :, :])
    nc.compile()
    return nc


def _get_nc():
    global _NC
    if _NC is None:
        _NC = _build()
    return _NC


def _run(inputs, trace=False, **kw):
    from concourse.bass_utils import run_bass_kernel_spmd

    delta = np.ascontiguousarray(
        np.asarray(inputs["delta"], dtype=np.float32).reshape(2, 1))
    u = np.ascontiguousarray(
        np.asarray(inputs["u"], dtype=np.float32).reshape(2, 1))
    assert int(inputs.get("seq_len", N)) == N
    assert int(inputs.get("k", KOFF)) == KOFF

    nc = _get_nc()
    tmat, bsel, tri = _host_consts()
    duv = np.ascontiguousarray(np.concatenate([delta, u], axis=0))
    in_maps = []
    for c in range(NCORES):
        in_maps.append({
            "duv": duv, "tmat": tmat, "bsel": bsel,
            "tri": tri, "wsel": _wsel_for_core(c),
        })
    res = run_bass_kernel_spmd(nc, in_maps, core_ids=list(range(NCORES)),
                               trace=trace, **kw)
    shards = [res.results[i]["out"] for i in range(NCORES)]
    full = np.concatenate(shards, axis=0)  # (2048, 16, 4096)
    return full[:, None, :, :], res


def kernel(**inputs) -> np.ndarray:
    out, _ = _run(inputs)
    return np.ascontiguousarray(out.astype(np.float32, copy=False))
